# revision 1
# baseline (speedup 1.0000x reference)
"""Trainium2 Bass kernel for BertSelfAttentionDistance.

Problem (per batch b, B=8, S=2048, H=1024, NH=1, DT=64):
    q = hs @ Wq.T + bq ; k = hs @ Wk.T + bk ; v = hs @ Wv.T + bv
    scores = q @ k.T
    wdd    = q @ dist_emb.T                       # [S, DT]
    bias   = take_along(wdd, rel, axis=-1) * (rel == 1)
    out    = softmax((scores + bias)/sqrt(H) + attn_mask) @ v

Key simplification: because the gathered value is multiplied by (rel == 1),
only wdd[:, 1] = q @ dist_emb[1] ever survives:
    bias[i, j] = (rel[i, j] == 1) * (q[i] . dist_emb[1])
so the [S, S] gather is never needed — just a compare and a broadcast.

Sharding: pure data-parallel, one batch per NeuronCore (8 batches, 8 cores).

Per-core layout (ST = transposed-scores layout, keys on partitions):
    QT[o, s], KT[o, s]  (via out = W.T stationary / hsT moving)
    ST[j, q] = sum_d KT[d, j] * QT[d, q]          (keys j on partitions)
    E[j, q]  = exp(ST/32 + (relT==1)*w1[q]/32 + am[j])   (unnormalized)
    denom[q] = ones[1,j] @ E    (PE reduction over partitions)
    ctxT[d, q] = (sum_j (V[j, d]+bv[d]) * E[j, q]) * (1/denom[q])

v bias note: softmax rows sum to 1, so ctx = P @ (V0 + 1*bv) = P@V0 + bv;
with unnormalized E: (E@(V0+1*bv)) / denom = ctx0 + bv exactly.
"""

import sys

sys.path.insert(0, "/opt/trn_rl_repo")

from contextlib import ExitStack

import ml_dtypes
import numpy as np

import concourse.bass as bass
import concourse.tile as tile
from concourse import bacc, mybir
from concourse._compat import with_exitstack
from concourse.bass_utils import run_bass_kernel_spmd

B, S, H, DT = 8, 2048, 1024, 64
NCORES = 8
P = 128
CHUNK = 512  # q-chunk = one fp32 PSUM bank
SCALE = 1.0 / 32.0  # 1/sqrt(H)

BF16 = mybir.dt.bfloat16
F32 = mybir.dt.float32
I32 = mybir.dt.int32
AF = mybir.ActivationFunctionType
ALU = mybir.AluOpType

NPBF16 = ml_dtypes.bfloat16


def _bcast_ap(src_row):
    """Partition-broadcast AP: read one [1, N] row as [128, N]."""
    return bass.AP(
        tensor=src_row.tensor,
        offset=src_row.offset,
        ap=[[0, P], list(src_row.ap[-1])],
    )


@with_exitstack
def _attn_kernel(ctx: ExitStack, tc: tile.TileContext, outs, ins, s=S):
    nc = tc.nc
    JT = s // P  # key tiles along sequence
    NCH = s // CHUNK  # query chunks
    HT = H // P  # 8
    HC = H // CHUNK  # 2

    hsT = ins["hsT"].rearrange("(t p) s -> p t s", p=P)  # [128, HT, s]
    wqT = ins["wqT"].rearrange("(t p) o -> p t o", p=P)  # [128, HT, H]
    wkT = ins["wkT"].rearrange("(t p) o -> p t o", p=P)
    wvT = ins["wvT"].rearrange("(t p) o -> p t o", p=P)
    relT = ins["relT"].rearrange("(t p) q -> p t q", p=P)  # [128, JT, s]
    ctxT = outs["ctxT"].rearrange("(t p) q -> p t q", p=P)  # [128, HT, s]

    consts = ctx.enter_context(tc.tile_pool(name="consts", bufs=1))
    qkv = ctx.enter_context(tc.tile_pool(name="qkv", bufs=1))

    # small per-partition constants
    bq_s = consts.tile([P, HT], F32)
    nc.sync.dma_start(out=bq_s, in_=ins["bq"])
    bk_s = consts.tile([P, HT], F32)
    nc.sync.dma_start(out=bk_s, in_=ins["bk"])
    d1_s = consts.tile([P, HT], BF16)
    nc.sync.dma_start(out=d1_s, in_=ins["d1"])
    am_s = consts.tile([P, JT], F32)
    nc.sync.dma_start(out=am_s, in_=ins["am"])
    # bv broadcast across partitions: [H] -> [128, H] (bf16: V is bf16 anyway)
    bvB = consts.tile([P, H], BF16)
    nc.sync.dma_start(out=bvB, in_=_bcast_ap(ins["bv"]))
    ones_bf = consts.tile([P, 1], BF16)
    nc.vector.memset(ones_bf, 1.0)
    # d1 pre-scaled by 1/32 (exact in bf16: power of two)
    d1_sc = consts.tile([P, HT], BF16)
    nc.vector.tensor_scalar_mul(d1_sc, d1_s, SCALE)

    # persistent bf16 operands for the attention stage
    QT = qkv.tile([P, HT, s], BF16)
    KT = qkv.tile([P, HT, s], BF16)
    V = qkv.tile([P, JT, H], BF16)
    W1B = qkv.tile([P, NCH, CHUNK], BF16)  # (q . d1)/32, partition-broadcast

    # scores-phase PSUM pools opened BEFORE stage A's pool so they land in
    # banks stage A never touches — chunk-0 scores can start while stage A
    # epilogues drain.
    ps_score = ctx.enter_context(tc.tile_pool(name="ps_s", bufs=4, space="PSUM"))
    ps_vec = ctx.enter_context(tc.tile_pool(name="ps_vec", bufs=1, space="PSUM"))
    dram_scratch = ctx.enter_context(
        tc.tile_pool(name="dscratch", bufs=2, space="DRAM")
    )
    # rel/bias pools outside the stage-A region so chunk-0's rel DMAs and
    # biasT writes don't WAR-wait on stage-A's hs/w tiles.
    relpool = ctx.enter_context(tc.tile_pool(name="rel", bufs=3))
    biaspool = ctx.enter_context(tc.tile_pool(name="bias", bufs=3))

    # ---- stage A: projections ----
    with (
        tc.tile_pool(name="stage_a", bufs=1) as sa,
        tc.tile_pool(name="psum_a", bufs=3, space="PSUM") as psa,
    ):
        hs_sb = sa.tile([P, HT, s], BF16)
        wq_sb = sa.tile([P, HT, H], BF16)
        wk_sb = sa.tile([P, HT, H], BF16)
        wv_sb = sa.tile([P, HT, H], BF16)
        # whole-row transfers (4KB/partition lines); first-consumed first:
        # QT's group 0 streams all of wq+hs, so give them the full HBM
        # bandwidth before wk/wv start competing.
        for t in range(HT):
            nc.sync.dma_start(out=wq_sb[:, t, :], in_=wqT[:, t, :])
            nc.sync.dma_start(out=hs_sb[:, t, :], in_=hsT[:, t, :])
        for t in range(HT):
            nc.sync.dma_start(out=wk_sb[:, t, :], in_=wkT[:, t, :])
            nc.sync.dma_start(out=wv_sb[:, t, :], in_=wvT[:, t, :])

        # Dummy matmuls: keep the PE busy (and the HAM clock-gate warm)
        # while the first wq/hs tiles stream in. Zero data, never consumed.
        dummy_src = sa.tile([P, 256], BF16)
        nc.vector.memset(dummy_src, 0.0)
        dummy_ps = ps_vec.tile([P, 256], F32, tag="vec1", name="dummy_ps")
        for _ in range(120):
            nc.tensor.matmul(
                dummy_ps,
                dummy_src[:, 0:P],
                dummy_src,
                start=True,
                stop=True,
                skip_group_check=True,
            )

        # QT[o, s] / KT[o, s] = sum_i W.T[i, o] * hsT[i, s]  (+ bias per o)
        def _proj(w_sb, b_s, out_sb):
            for ot in range(HT):
                for c0 in range(0, NCH, 2):
                    cpair = tuple(c for c in (c0, c0 + 1) if c < NCH)
                    pss = [
                        psa.tile([P, CHUNK], F32, tag="pa", name=f"pa_{i}")
                        for i in range(len(cpair))
                    ]
                    for it in range(HT):
                        for i, c in enumerate(cpair):
                            nc.tensor.matmul(
                                pss[i],
                                w_sb[:, it, ot * P : (ot + 1) * P],
                                hs_sb[:, it, c * CHUNK : (c + 1) * CHUNK],
                                start=(it == 0),
                                stop=(it == HT - 1),
                            )
                    for i, c in enumerate(cpair):
                        nc.scalar.activation(
                            out_sb[:, ot, c * CHUNK : (c + 1) * CHUNK],
                            pss[i],
                            AF.Identity,
                            bias=b_s[:, ot : ot + 1],
                        )

        _proj(wq_sb, bq_s, QT)

        # W1B precompute: w1[q] = q[q] . d1/32, all chunks, col-tiled 4x.
        # Emitted right after QT so the DRAM-broadcast roundtrip completes
        # long before chunk 0 needs it.
        w1p = ps_vec.tile([P, CHUNK], F32, tag="vec1")
        for kt in range(HT):
            for c in range(NCH):
                nc.tensor.matmul(
                    w1p[32 * c : 32 * c + 1, :],
                    d1_sc[:, kt : kt + 1],
                    QT[:, kt, c * CHUNK : (c + 1) * CHUNK],
                    start=(kt == 0),
                    stop=(kt == HT - 1),
                    tile_position=(0, 32 * c),
                    skip_group_check=True,
                )
        w1rows = consts.tile([1, NCH, CHUNK], BF16)
        w1d = dram_scratch.tile([1, NCH, CHUNK], BF16)
        for c in range(NCH):
            nc.vector.tensor_copy(w1rows[:, c, :], w1p[32 * c : 32 * c + 1, :])
            nc.sync.dma_start(out=w1d[:, c, :], in_=w1rows[:, c, :])
            nc.sync.dma_start(out=W1B[:, c, :], in_=_bcast_ap(w1d[:, c, :]))

        _proj(wk_sb, bk_s, KT)

        # V[j, o] = sum_i hsT[i, j] * WvT[i, o] + bv[o].  V last: it has no
        # chunk-0 consumers until PV, so its matmuls give the scheduler PE
        # filler while chunk-0's softmax pipeline warms up.
        for jt in range(JT):
            pss = [
                psa.tile([P, CHUNK], F32, tag="pa", name=f"pav_{i}")
                for i in range(HC)
            ]
            for it in range(HT):
                for oc in range(HC):
                    nc.tensor.matmul(
                        pss[oc],
                        hs_sb[:, it, jt * P : (jt + 1) * P],
                        wv_sb[:, it, oc * CHUNK : (oc + 1) * CHUNK],
                        start=(it == 0),
                        stop=(it == HT - 1),
                    )
            for oc in range(HC):
                nc.vector.tensor_tensor(
                    V[:, jt, oc * CHUNK : (oc + 1) * CHUNK],
                    pss[oc],
                    bvB[:, oc * CHUNK : (oc + 1) * CHUNK],
                    op=ALU.add,
                )

    # ---- stage B pools ----
    epool = ctx.enter_context(tc.tile_pool(name="E", bufs=2))
    recpool = ctx.enter_context(tc.tile_pool(name="rec", bufs=2))
    outpool = ctx.enter_context(tc.tile_pool(name="out", bufs=3))
    ps_pv = ctx.enter_context(tc.tile_pool(name="ps_pv", bufs=3, space="PSUM"))

    # ---- stage B: per query chunk ----
    # PV for chunk c is emitted after the scores/softmax of chunk c+1, so
    # the denom->reciprocal->broadcast chain of chunk c overlaps an entire
    # scores phase and the PE never waits on it.
    deferred_pv = []

    def emit_pv(c, E, recB):
        cs = slice(c * CHUNK, (c + 1) * CHUNK)
        for dt in range(HT):
            ps2 = ps_pv.tile([P, CHUNK], F32)
            for jt in range(JT):
                nc.tensor.matmul(
                    ps2,
                    V[:, jt, dt * P : (dt + 1) * P],
                    E[:, jt, :],
                    start=(jt == 0),
                    stop=(jt == JT - 1),
                )
            ot_t = outpool.tile([P, CHUNK], F32)
            nc.vector.tensor_tensor(ot_t, ps2, recB, op=ALU.mult)
            nc.sync.dma_start(out=ctxT[:, dt, cs], in_=ot_t)

    for c in range(NCH):
        cs = slice(c * CHUNK, (c + 1) * CHUNK)

        E = epool.tile([P, JT, CHUNK], BF16)
        dps = ps_vec.tile([P, CHUNK], F32, tag="vec1")

        def denom_pair(jt0):
            # two adjacent col-tiled [128,1] matmuls overlap on the PE
            for i, jj in enumerate((jt0, jt0 + 1)):
                nc.tensor.matmul(
                    dps[32 * i : 32 * i + 1, :],
                    ones_bf,
                    E[:, jj, :],
                    start=(jj < 2),
                    stop=(jj >= JT - 2),
                    tile_position=(0, 32 * i),
                    skip_group_check=True,
                )

        for jt in range(JT):
            rel_t = relpool.tile([P, CHUNK], I32)
            nc.sync.dma_start(out=rel_t, in_=relT[:, jt, cs])
            biasT = biaspool.tile([P, CHUNK], BF16)
            nc.vector.scalar_tensor_tensor(
                biasT, rel_t, 1, W1B[:, c, :], op0=ALU.is_equal, op1=ALU.mult
            )
            ps = ps_score.tile([P, CHUNK], F32)
            for dt in range(HT):
                nc.tensor.matmul(
                    ps,
                    KT[:, dt, jt * P : (jt + 1) * P],
                    QT[:, dt, cs],
                    start=(dt == 0),
                    stop=(dt == HT - 1),
                )
            nc.vector.scalar_tensor_tensor(
                ps, ps, SCALE, biasT, op0=ALU.mult, op1=ALU.add
            )
            nc.scalar.activation(E[:, jt, :], ps, AF.Exp, bias=am_s[:, jt : jt + 1])
            # interleave denominator accumulation a few tiles behind
            if jt >= 3 and jt % 2 == 1:
                denom_pair(jt - 3)
        denom_pair(JT - 2)

        # denom rows 0 + 32 -> sum -> broadcast -> reciprocal
        drow = recpool.tile([1, CHUNK], F32, tag="drow")
        nc.vector.tensor_copy(drow, dps[32:33, :])
        dsum = recpool.tile([1, CHUNK], F32, tag="dsum")
        nc.vector.tensor_tensor(dsum, dps[0:1, :], drow, op=ALU.add)
        dsum_d = dram_scratch.tile([1, CHUNK], F32, tag="dsum_d")
        nc.sync.dma_start(out=dsum_d, in_=dsum)
        denB = recpool.tile([P, CHUNK], F32, tag="denB")
        nc.sync.dma_start(out=denB, in_=_bcast_ap(dsum_d))
        recB = recpool.tile([P, CHUNK], F32, tag="recB")
        rscr = recpool.tile([P, CHUNK], F32, tag="rscr")
        nc.vector.reciprocal_approx_accurate(recB, denB, rscr)

        deferred_pv.append((c, E, recB))
        if len(deferred_pv) > 1:
            emit_pv(*deferred_pv.pop(0))
    while deferred_pv:
        emit_pv(*deferred_pv.pop(0))


def build_program(s=S):
    """Build + compile the per-core Bass program."""
    JT = s // P
    HT = H // P
    nc = bacc.Bacc("TRN2", target_bir_lowering=False, debug=False)
    ins = {
        "hsT": nc.dram_tensor("hsT", [H, s], BF16, kind="ExternalInput").ap(),
        "wqT": nc.dram_tensor("wqT", [H, H], BF16, kind="ExternalInput").ap(),
        "wkT": nc.dram_tensor("wkT", [H, H], BF16, kind="ExternalInput").ap(),
        "wvT": nc.dram_tensor("wvT", [H, H], BF16, kind="ExternalInput").ap(),
        "bq": nc.dram_tensor("bq", [P, HT], F32, kind="ExternalInput").ap(),
        "bk": nc.dram_tensor("bk", [P, HT], F32, kind="ExternalInput").ap(),
        "bv": nc.dram_tensor("bv", [1, H], BF16, kind="ExternalInput").ap(),
        "d1": nc.dram_tensor("d1", [P, HT], BF16, kind="ExternalInput").ap(),
        "am": nc.dram_tensor("am", [P, JT], F32, kind="ExternalInput").ap(),
        "relT": nc.dram_tensor("relT", [s, s], I32, kind="ExternalInput").ap(),
    }
    outs = {
        "ctxT": nc.dram_tensor("ctxT", [H, s], F32, kind="ExternalOutput").ap(),
    }
    with tile.TileContext(nc) as tc:
        _attn_kernel(tc, outs, ins, s=s)
    nc.compile()
    return nc


def make_in_maps(
    hidden_states,
    attention_mask,
    word_word_relation,
    Wq,
    bq,
    Wk,
    bk,
    Wv,
    bv,
    dist_emb,
    s=S,
):
    """Host-side sharding/layout marshalling: one batch per core."""
    HT = H // P
    JT = s // P
    hs = np.asarray(hidden_states, dtype=np.float32)
    am = np.asarray(attention_mask, dtype=np.float32)
    rel = np.ascontiguousarray(np.asarray(word_word_relation, dtype=np.int32))
    wqT = np.ascontiguousarray(np.asarray(Wq, np.float32).T.astype(NPBF16))
    wkT = np.ascontiguousarray(np.asarray(Wk, np.float32).T.astype(NPBF16))
    wvT = np.ascontiguousarray(np.asarray(Wv, np.float32).T.astype(NPBF16))
    bq_s = np.ascontiguousarray(np.asarray(bq, np.float32).reshape(HT, P).T)
    bk_s = np.ascontiguousarray(np.asarray(bk, np.float32).reshape(HT, P).T)
    bv_s = np.ascontiguousarray(np.asarray(bv, np.float32).astype(NPBF16).reshape(1, H))
    d1_s = np.ascontiguousarray(
        np.asarray(dist_emb, np.float32)[1].astype(NPBF16).reshape(HT, P).T
    )
    in_maps = []
    for b in range(hs.shape[0]):
        hsT = np.ascontiguousarray(hs[b].T.astype(NPBF16))
        relT = np.ascontiguousarray(rel[b].T)
        am_s = np.ascontiguousarray(am[b, 0, 0].reshape(JT, P).T)
        in_maps.append(
            {
                "hsT": hsT,
                "wqT": wqT,
                "wkT": wkT,
                "wvT": wvT,
                "bq": bq_s,
                "bk": bk_s,
                "bv": bv_s,
                "d1": d1_s,
                "am": am_s,
                "relT": relT,
            }
        )
    return in_maps


_NC_CACHE = {}


def get_program(s=S):
    if s not in _NC_CACHE:
        _NC_CACHE[s] = build_program(s)
    return _NC_CACHE[s]


def run(inputs: dict, trace: bool = False):
    """Run on hardware; returns (output [B,S,H] f32, BassKernelResults)."""
    nc = get_program(S)
    in_maps = make_in_maps(**inputs)
    res = run_bass_kernel_spmd(nc, in_maps, list(range(NCORES)), trace=trace)
    out = np.stack(
        [np.ascontiguousarray(r["ctxT"].T) for r in res.results], axis=0
    ).astype(np.float32)
    return out, res


def kernel(**inputs) -> np.ndarray:
    out, _ = run(inputs, trace=False)
    return out



# revision 2
# speedup vs baseline: 1.3713x; 1.3713x over previous
"""Trainium2 Bass kernel for BertSelfAttentionDistance.

Problem (per batch b, B=8, S=2048, H=1024, NH=1, DT=64):
    q = hs @ Wq.T + bq ; k = hs @ Wk.T + bk ; v = hs @ Wv.T + bv
    scores = q @ k.T
    wdd    = q @ dist_emb.T                       # [S, DT]
    bias   = take_along(wdd, rel, axis=-1) * (rel == 1)
    out    = softmax((scores + bias)/sqrt(H) + attn_mask) @ v

Key simplifications:
1. Because the gathered value is multiplied by (rel == 1), only
   wdd[:, 1] = q @ dist_emb[1] survives:
       bias[i, j] = (rel[i, j] == 1) * (q[i] . dist_emb[1])
   so the [S, S] gather is never needed — just a compare and broadcast.
2. q and k are never needed individually — only scores and w1:
       scores0 = hs @ (Wq.T @ Wk) @ hs.T = G @ hs.T,   G = hs @ M
       w1      = hs @ (Wq.T @ dist_emb[1]) (+ bq . dist_emb[1])
   M = Wq.T @ Wk and m1 = Wq.T @ d1 are folded on the host (weight-only,
   O(H^2) preprocessing); this removes one full [S,H]x[H,H] projection
   per core and the hs tile doubles as the scores stationary.
   Bias terms: q0.bk and bq.bk are constant per query row -> softmax-
   invariant, dropped exactly. bq.k0[k] varies per key: handled by a
   conditionally-compiled kb path (has_bq) feeding the exp bias; the
   benchmark has bq = 0 so the lean variant is used.

Sharding: pure data-parallel, one batch per NeuronCore (8 batches, 8 cores).

Per-core layout (ST = transposed-scores layout, keys on partitions):
    GT[d, s]  (via M stationary / hsT moving)
    ST[j, q] = sum_d hsT[d, j] * GT[d, q]         (keys j on partitions)
    E[j, q]  = exp(ST/32 + (relT==1)*w1[q]/32 + am[j])   (unnormalized)
    denom[q] = ones[1,j] @ E    (PE reduction over partitions)
    ctxT[d, q] = (sum_j (V[j, d]+bv[d]) * E[j, q]) * (1/denom[q])

v bias note: softmax rows sum to 1, so ctx = P @ (V0 + 1*bv) = P@V0 + bv;
with unnormalized E: (E@(V0+1*bv)) / denom = ctx0 + bv exactly.
"""

import sys

sys.path.insert(0, "/opt/trn_rl_repo")

from contextlib import ExitStack

import ml_dtypes
import numpy as np

import concourse.bass as bass
import concourse.tile as tile
from concourse import bacc, mybir
from concourse._compat import with_exitstack
from concourse.bass_utils import run_bass_kernel_spmd

B, S, H, DT = 8, 2048, 1024, 64
NCORES = 8
P = 128
CHUNK = 512  # q-chunk = one fp32 PSUM bank
SCALE = 1.0 / 32.0  # 1/sqrt(H)
NDUMMY = 60

BF16 = mybir.dt.bfloat16
F32 = mybir.dt.float32
I32 = mybir.dt.int32
AF = mybir.ActivationFunctionType
ALU = mybir.AluOpType

NPBF16 = ml_dtypes.bfloat16


def _bcast_ap(src_row):
    """Partition-broadcast AP: read one [1, N] row as [128, N]."""
    return bass.AP(
        tensor=src_row.tensor,
        offset=src_row.offset,
        ap=[[0, P], list(src_row.ap[-1])],
    )


@with_exitstack
def _attn_kernel(ctx: ExitStack, tc: tile.TileContext, outs, ins, s=S, has_bq=False):
    nc = tc.nc
    JT = s // P  # key tiles along sequence
    NCH = s // CHUNK  # query chunks
    HT = H // P  # 8
    HC = H // CHUNK  # 2

    hsT = ins["hsT"].rearrange("(t p) s -> p t s", p=P)  # [128, HT, s]
    mT = ins["m"].rearrange("(t p) o -> p t o", p=P)  # [128, HT, H]
    wvT = ins["wvT"].rearrange("(t p) o -> p t o", p=P)
    relT = ins["relT"].rearrange("(t p) q -> p t q", p=P)  # [128, JT, s]
    ctxT = outs["ctxT"].rearrange("(t p) q -> p t q", p=P)  # [128, HT, s]

    consts = ctx.enter_context(tc.tile_pool(name="consts", bufs=1))
    qkv = ctx.enter_context(tc.tile_pool(name="qkv", bufs=1))

    # small per-partition constants
    m1_s = consts.tile([P, HT], BF16)
    nc.sync.dma_start(out=m1_s, in_=ins["m1"])
    am_s = consts.tile([P, JT], F32)
    nc.sync.dma_start(out=am_s, in_=ins["am"])
    # bv broadcast across partitions: [H] -> [128, H]
    bvB = consts.tile([P, H], BF16)
    nc.sync.dma_start(out=bvB, in_=_bcast_ap(ins["bv"]))
    ones_bf = consts.tile([P, 1], BF16)
    nc.vector.memset(ones_bf, 1.0)
    if has_bq:
        mb_s = consts.tile([P, HT], BF16)
        nc.sync.dma_start(out=mb_s, in_=ins["mb"])
        c_s = consts.tile([1, 1], F32)
        nc.sync.dma_start(out=c_s, in_=ins["c"])

    # persistent bf16 operands for the attention stage.  hs_sb stays
    # resident: it is the moving operand of G/w1 and the stationary of
    # scores and the V projection.
    hs_sb = qkv.tile([P, HT, s], BF16)
    GT = qkv.tile([P, HT, s], BF16)
    V = qkv.tile([P, JT, H], BF16)
    W1B = qkv.tile([P, NCH, CHUNK], BF16)  # (q . d1)/32, partition-broadcast

    # scores-phase PSUM pools opened BEFORE stage A's pool so they land in
    # banks stage A never touches — chunk-0 scores can start while stage A
    # epilogues drain.
    ps_score = ctx.enter_context(tc.tile_pool(name="ps_s", bufs=3, space="PSUM"))
    ps_vec = ctx.enter_context(tc.tile_pool(name="ps_vec", bufs=1, space="PSUM"))
    dram_scratch = ctx.enter_context(
        tc.tile_pool(name="dscratch", bufs=3 if has_bq else 2, space="DRAM")
    )
    # rel/bias pools outside the stage-A region so chunk-0's rel DMAs and
    # biasT writes don't WAR-wait on stage-A's hs/w tiles.
    relpool = ctx.enter_context(tc.tile_pool(name="rel", bufs=3))
    biaspool = ctx.enter_context(tc.tile_pool(name="bias", bufs=3))

    # ---- stage A: G projection, w1, V ----
    with (
        tc.tile_pool(name="stage_a", bufs=1) as sa,
        tc.tile_pool(name="psum_a", bufs=3, space="PSUM") as psa,
    ):
        m_sb = sa.tile([P, HT, H], BF16)
        wv_sb = sa.tile([P, HT, H], BF16)
        # First-consumed first: G's group (ot=0, chunks 0-1) needs the
        # first 256 M columns and the first hs half, so stream those
        # before the rest competes for HBM bandwidth.
        for t in range(HT):
            nc.sync.dma_start(out=m_sb[:, t, 0:256], in_=mT[:, t, 0:256])
            nc.sync.dma_start(out=hs_sb[:, t, 0 : s // 2], in_=hsT[:, t, 0 : s // 2])
        for t in range(HT):
            nc.sync.dma_start(out=m_sb[:, t, 256:H], in_=mT[:, t, 256:H])
            nc.sync.dma_start(
                out=hs_sb[:, t, s // 2 : s], in_=hsT[:, t, s // 2 : s]
            )
        for t in range(HT):
            nc.sync.dma_start(out=wv_sb[:, t, :], in_=wvT[:, t, :])

        # Dummy matmuls: keep the PE busy (and the HAM clock-gate warm)
        # while the first m/hs tiles stream in. Zero data, never consumed.
        dummy_src = sa.tile([P, 256], BF16)
        nc.vector.memset(dummy_src, 0.0)
        dummy_ps = ps_vec.tile([P, 256], F32, tag="vec1", name="dummy_ps")
        for _ in range(NDUMMY):
            nc.tensor.matmul(
                dummy_ps,
                dummy_src[:, 0:P],
                dummy_src,
                start=True,
                stop=True,
                skip_group_check=True,
            )

        # GT[o, s] = sum_i M[i, o] * hsT[i, s]
        for ot in range(HT):
            for c0 in range(0, NCH, 2):
                cpair = tuple(c for c in (c0, c0 + 1) if c < NCH)
                pss = [
                    psa.tile([P, CHUNK], F32, tag="pa", name=f"pa_{i}")
                    for i in range(len(cpair))
                ]
                for it in range(HT):
                    for i, c in enumerate(cpair):
                        nc.tensor.matmul(
                            pss[i],
                            m_sb[:, it, ot * P : (ot + 1) * P],
                            hs_sb[:, it, c * CHUNK : (c + 1) * CHUNK],
                            start=(it == 0),
                            stop=(it == HT - 1),
                        )
                for i, c in enumerate(cpair):
                    nc.scalar.activation(
                        GT[:, ot, c * CHUNK : (c + 1) * CHUNK],
                        pss[i],
                        AF.Identity,
                        bias=0.0,
                    )

        # W1B precompute: w1[q]/32 = hs[q] . m1/32, all chunks, col-tiled 4x.
        # Emitted right after G so the DRAM-broadcast roundtrip completes
        # long before chunk 0 needs it.
        w1p = ps_vec.tile([P, CHUNK], F32, tag="vec1")
        for it in range(HT):
            for c in range(NCH):
                nc.tensor.matmul(
                    w1p[32 * c : 32 * c + 1, :],
                    m1_s[:, it : it + 1],
                    hs_sb[:, it, c * CHUNK : (c + 1) * CHUNK],
                    start=(it == 0),
                    stop=(it == HT - 1),
                    tile_position=(0, 32 * c),
                    skip_group_check=True,
                )
        w1rows = consts.tile([1, NCH, CHUNK], BF16)
        w1d = dram_scratch.tile([1, NCH, CHUNK], BF16)
        for c in range(NCH):
            if has_bq:
                # w1_full/32 = hs.m1/32 + (bq.d1)/32
                nc.scalar.activation(
                    w1rows[:, c, :],
                    w1p[32 * c : 32 * c + 1, :],
                    AF.Identity,
                    bias=c_s[0:1, 0:1],
                )
            else:
                nc.vector.tensor_copy(w1rows[:, c, :], w1p[32 * c : 32 * c + 1, :])
            nc.sync.dma_start(out=w1d[:, c, :], in_=w1rows[:, c, :])
            nc.sync.dma_start(out=W1B[:, c, :], in_=_bcast_ap(w1d[:, c, :]))

        if has_bq:
            # kb[k]/32 = hs[k] . (Wk.T bq)/32, added to the per-key exp bias.
            kbp = ps_vec.tile([P, CHUNK], F32, tag="vec1")
            for it in range(HT):
                for c in range(NCH):
                    nc.tensor.matmul(
                        kbp[32 * c : 32 * c + 1, :],
                        mb_s[:, it : it + 1],
                        hs_sb[:, it, c * CHUNK : (c + 1) * CHUNK],
                        start=(it == 0),
                        stop=(it == HT - 1),
                        tile_position=(0, 32 * c),
                        skip_group_check=True,
                    )
            kbrow = consts.tile([1, NCH, CHUNK], F32)
            for c in range(NCH):
                nc.vector.tensor_copy(kbrow[:, c, :], kbp[32 * c : 32 * c + 1, :])
            kbd = dram_scratch.tile([1, NCH, CHUNK], F32)
            nc.sync.dma_start(out=kbd, in_=kbrow)
            kb_s = consts.tile([P, JT], F32)
            nc.sync.dma_start(
                out=kb_s,
                in_=bass.AP(tensor=kbd.tensor, offset=kbd.offset, ap=[[1, P], [P, JT]]),
            )
            am_eff = consts.tile([P, JT], F32)
            nc.vector.tensor_tensor(am_eff, am_s, kb_s, op=ALU.add)
            am_x = am_eff
        else:
            am_x = am_s

        # V[j, o] = sum_i hsT[i, j] * WvT[i, o] + bv[o].  V last: it has no
        # chunk-0 consumers until PV, so its matmuls give the scheduler PE
        # filler while chunk-0's softmax pipeline warms up.
        for jt in range(JT):
            pss = [
                psa.tile([P, CHUNK], F32, tag="pa", name=f"pav_{i}")
                for i in range(HC)
            ]
            for it in range(HT):
                for oc in range(HC):
                    nc.tensor.matmul(
                        pss[oc],
                        hs_sb[:, it, jt * P : (jt + 1) * P],
                        wv_sb[:, it, oc * CHUNK : (oc + 1) * CHUNK],
                        start=(it == 0),
                        stop=(it == HT - 1),
                    )
            for oc in range(HC):
                nc.vector.tensor_tensor(
                    V[:, jt, oc * CHUNK : (oc + 1) * CHUNK],
                    pss[oc],
                    bvB[:, oc * CHUNK : (oc + 1) * CHUNK],
                    op=ALU.add,
                )

    # ---- stage B pools ----
    epool = ctx.enter_context(tc.tile_pool(name="E", bufs=2))
    recpool = ctx.enter_context(tc.tile_pool(name="rec", bufs=2))
    outpool = ctx.enter_context(tc.tile_pool(name="out", bufs=3))
    ps_pv = ctx.enter_context(tc.tile_pool(name="ps_pv", bufs=4, space="PSUM"))

    # ---- stage B: per query chunk ----
    # PV for chunk c is emitted after the scores/softmax of chunk c+1, so
    # the denom->reciprocal->broadcast chain of chunk c overlaps an entire
    # scores phase and the PE never waits on it.
    deferred_pv = []

    def emit_pv(c, E, recB):
        cs = slice(c * CHUNK, (c + 1) * CHUNK)
        for dt in range(HT):
            ps2 = ps_pv.tile([P, CHUNK], F32)
            for jt in range(JT):
                nc.tensor.matmul(
                    ps2,
                    V[:, jt, dt * P : (dt + 1) * P],
                    E[:, jt, :],
                    start=(jt == 0),
                    stop=(jt == JT - 1),
                )
            ot_t = outpool.tile([P, CHUNK], F32)
            nc.vector.tensor_tensor(ot_t, ps2, recB, op=ALU.mult)
            nc.sync.dma_start(out=ctxT[:, dt, cs], in_=ot_t)

    for c in range(NCH):
        cs = slice(c * CHUNK, (c + 1) * CHUNK)

        E = epool.tile([P, JT, CHUNK], BF16)
        dps = ps_vec.tile([P, CHUNK], F32, tag="vec1")

        def denom_pair(jt0):
            # two adjacent col-tiled [128,1] matmuls overlap on the PE
            for i, jj in enumerate((jt0, jt0 + 1)):
                nc.tensor.matmul(
                    dps[32 * i : 32 * i + 1, :],
                    ones_bf,
                    E[:, jj, :],
                    start=(jj < 2),
                    stop=(jj >= JT - 2),
                    tile_position=(0, 32 * i),
                    skip_group_check=True,
                )

        for jt in range(JT):
            rel_t = relpool.tile([P, CHUNK], I32)
            nc.sync.dma_start(out=rel_t, in_=relT[:, jt, cs])
            biasT = biaspool.tile([P, CHUNK], BF16)
            nc.vector.scalar_tensor_tensor(
                biasT, rel_t, 1, W1B[:, c, :], op0=ALU.is_equal, op1=ALU.mult
            )
            ps = ps_score.tile([P, CHUNK], F32)
            for dt in range(HT):
                nc.tensor.matmul(
                    ps,
                    hs_sb[:, dt, jt * P : (jt + 1) * P],
                    GT[:, dt, cs],
                    start=(dt == 0),
                    stop=(dt == HT - 1),
                )
            nc.vector.scalar_tensor_tensor(
                ps, ps, SCALE, biasT, op0=ALU.mult, op1=ALU.add
            )
            nc.scalar.activation(E[:, jt, :], ps, AF.Exp, bias=am_x[:, jt : jt + 1])
            # interleave denominator accumulation a few tiles behind
            if jt >= 3 and jt % 2 == 1:
                denom_pair(jt - 3)
        denom_pair(JT - 2)

        # denom rows 0 + 32 -> sum -> broadcast -> reciprocal
        drow = recpool.tile([1, CHUNK], F32, tag="drow")
        nc.vector.tensor_copy(drow, dps[32:33, :])
        dsum = recpool.tile([1, CHUNK], F32, tag="dsum")
        nc.vector.tensor_tensor(dsum, dps[0:1, :], drow, op=ALU.add)
        dsum_d = dram_scratch.tile([1, CHUNK], F32, tag="dsum_d")
        nc.sync.dma_start(out=dsum_d, in_=dsum)
        denB = recpool.tile([P, CHUNK], F32, tag="denB")
        nc.sync.dma_start(out=denB, in_=_bcast_ap(dsum_d))
        recB = recpool.tile([P, CHUNK], F32, tag="recB")
        rscr = recpool.tile([P, CHUNK], F32, tag="rscr")
        nc.vector.reciprocal_approx_accurate(recB, denB, rscr)

        deferred_pv.append((c, E, recB))
        if len(deferred_pv) > 1:
            emit_pv(*deferred_pv.pop(0))
    while deferred_pv:
        emit_pv(*deferred_pv.pop(0))


def build_program(s=S, has_bq=False):
    """Build + compile the per-core Bass program."""
    JT = s // P
    HT = H // P
    nc = bacc.Bacc("TRN2", target_bir_lowering=False, debug=False)
    ins = {
        "hsT": nc.dram_tensor("hsT", [H, s], BF16, kind="ExternalInput").ap(),
        "m": nc.dram_tensor("m", [H, H], BF16, kind="ExternalInput").ap(),
        "wvT": nc.dram_tensor("wvT", [H, H], BF16, kind="ExternalInput").ap(),
        "m1": nc.dram_tensor("m1", [P, HT], BF16, kind="ExternalInput").ap(),
        "bv": nc.dram_tensor("bv", [1, H], BF16, kind="ExternalInput").ap(),
        "am": nc.dram_tensor("am", [P, JT], F32, kind="ExternalInput").ap(),
        "relT": nc.dram_tensor("relT", [s, s], I32, kind="ExternalInput").ap(),
    }
    if has_bq:
        ins["mb"] = nc.dram_tensor("mb", [P, HT], BF16, kind="ExternalInput").ap()
        ins["c"] = nc.dram_tensor("c", [1, 1], F32, kind="ExternalInput").ap()
    outs = {
        "ctxT": nc.dram_tensor("ctxT", [H, s], F32, kind="ExternalOutput").ap(),
    }
    with tile.TileContext(nc) as tc:
        _attn_kernel(tc, outs, ins, s=s, has_bq=has_bq)
    nc.compile()
    return nc


def make_in_maps(
    hidden_states,
    attention_mask,
    word_word_relation,
    Wq,
    bq,
    Wk,
    bk,
    Wv,
    bv,
    dist_emb,
    s=S,
):
    """Host-side sharding/layout marshalling: one batch per core.

    Weight-only folds (O(H^2), batch-independent): M = Wq.T @ Wk,
    m1 = Wq.T @ dist_emb[1].  bk only enters softmax-invariant terms.
    """
    HT = H // P
    JT = s // P
    hs = np.asarray(hidden_states, dtype=np.float32)
    am = np.asarray(attention_mask, dtype=np.float32)
    rel = np.ascontiguousarray(np.asarray(word_word_relation, dtype=np.int32))
    Wqf = np.asarray(Wq, np.float32)
    Wkf = np.asarray(Wk, np.float32)
    Wvf = np.asarray(Wv, np.float32)
    d1 = np.asarray(dist_emb, np.float32)[1]
    m_h = np.ascontiguousarray((Wqf.T @ Wkf).astype(NPBF16))
    m1_h = np.ascontiguousarray(
        ((Wqf.T @ d1) * SCALE).reshape(HT, P).T.astype(NPBF16)
    )
    wvT = np.ascontiguousarray(Wvf.T.astype(NPBF16))
    bv_s = np.ascontiguousarray(np.asarray(bv, np.float32).astype(NPBF16).reshape(1, H))
    bqf = np.asarray(bq, np.float32)
    has_bq = bool(np.any(bqf))
    if has_bq:
        mb_h = np.ascontiguousarray(
            ((Wkf.T @ bqf) * SCALE).reshape(HT, P).T.astype(NPBF16)
        )
        c_h = np.ascontiguousarray(
            np.array([[float(bqf @ d1) * SCALE]], dtype=np.float32)
        )
    in_maps = []
    for b in range(hs.shape[0]):
        hsT = np.ascontiguousarray(hs[b].T.astype(NPBF16))
        relT = np.ascontiguousarray(rel[b].T)
        am_s = np.ascontiguousarray(am[b, 0, 0].reshape(JT, P).T)
        im = {
            "hsT": hsT,
            "m": m_h,
            "wvT": wvT,
            "m1": m1_h,
            "bv": bv_s,
            "am": am_s,
            "relT": relT,
        }
        if has_bq:
            im["mb"] = mb_h
            im["c"] = c_h
        in_maps.append(im)
    return in_maps, has_bq


_NC_CACHE = {}


def get_program(s=S, has_bq=False):
    key = (s, has_bq)
    if key not in _NC_CACHE:
        _NC_CACHE[key] = build_program(s, has_bq)
    return _NC_CACHE[key]


def run(inputs: dict, trace: bool = False):
    """Run on hardware; returns (output [B,S,H] f32, BassKernelResults)."""
    in_maps, has_bq = make_in_maps(**inputs)
    nc = get_program(S, has_bq)
    res = run_bass_kernel_spmd(nc, in_maps, list(range(NCORES)), trace=trace)
    out = np.stack(
        [np.ascontiguousarray(r["ctxT"].T) for r in res.results], axis=0
    ).astype(np.float32)
    return out, res


def kernel(**inputs) -> np.ndarray:
    out, _ = run(inputs, trace=False)
    return out
